# revision 27
# baseline (speedup 1.0000x reference)
"""BitLinear (BitNet b1.58-style) kernel for Trainium2, 8-core SPMD.

Reference computation (fp32):
    scale_w = max(mean(|W|), EPS)                       # scalar over all of W
    dq_w    = clip(round(W / scale_w), -1, 1) * scale_w # ternary weight
    amax_t  = max(max_j |x[t, j]|, EPS)                 # per token
    s_t     = 127 / amax_t
    dq_x    = round(x * s_t) / s_t                      # 8-bit absmax act quant
    out     = dq_x @ dq_w.T + b

Device strategy (data-parallel over tokens):
  * x is flattened to [8192, 4096] tokens and sharded 8 ways (1024/core).
  * W is transposed on the host once (layout choice) so each core can
    stream W.T = [in, out] tiles with K on partitions, and replicated.
  * The |W| mean reduction is sharded 8 ways (each core reduces 512 rows
    of W) and combined with a 4-byte AllReduce.
  * The matmul runs in bf16 on the PE array: q_x in [-127,127] and
    q_w in {-1,0,1} are bf16-exact, and fp32 PSUM accumulation of 4096
    products bounded by 127 is exact (< 2^24). The scales are folded
    into a per-token multiplier applied on PSUM eviction:
        out[t, :] = (q_x @ q_w.T)[t, :] * (scale_w * amax_t / 127) + b
  * Rounding uses the fp32 magic-constant trick (v + 1.5*2^23) - 1.5*2^23,
    round-to-nearest-even, matching jnp.round.  For W the multiply by
    1/scale_w and the +C rounding are SEPARATE instructions (ACT then
    DVE) so the double rounding matches the reference's fl(W/scale)
    then round() exactly; a fused FMA would single-round and flip a few
    ternary weights at the 0.5/1.5 boundaries.
"""

import numpy as np

from concourse import bacc, bass_isa, masks, mybir, tile
from concourse.tile import add_dep_helper
from concourse.bass_utils import run_bass_kernel_spmd

F32 = mybir.dt.float32
BF16 = mybir.dt.bfloat16
AX = mybir.AxisListType
OP = mybir.AluOpType
AF = mybir.ActivationFunctionType

EPS = 1e-6
QMAX = 127.0
C_MAGIC = 1.5 * 2.0**23  # fp32 RNE rounding constant

N_CORES = 8
B, S, D_IN, D_OUT = 4, 2048, 4096, 4096
T_FULL = B * S  # 8192 tokens
T = T_FULL // N_CORES  # 1024 tokens per core
SR = D_OUT // N_CORES  # 512 W rows per core for the |W| mean


def build_bass(t=T, di=D_IN, do=D_OUT, sr=SR, n_cores=N_CORES):
    """Emit the per-core SPMD program. All cores run the same code on
    their own shard; the only cross-core op is a 4-byte AllReduce."""
    assert t % 128 == 0 and di % 512 == 0 and do % 512 == 0 and sr % 128 == 0
    mt = t // 128  # token tiles
    kt = di // 128  # contraction tiles
    nt = do // 512  # output-column blocks
    assert mt <= 8, "one PSUM bank per token tile"

    nc = bacc.Bacc(None)
    xs_d = nc.dram_tensor("xs", [t, di], F32, kind="ExternalInput")
    wt_d = nc.dram_tensor("wt", [di, do], F32, kind="ExternalInput")
    ws_d = nc.dram_tensor("wshard", [sr, di], F32, kind="ExternalInput")
    b_d = nc.dram_tensor("bias", [1, do], F32, kind="ExternalInput")
    out_d = nc.dram_tensor("out", [t, do], F32, kind="ExternalOutput")
    probe_d = nc.dram_tensor("probe", [1, 8], F32, kind="ExternalOutput")

    with tile.TileContext(nc) as tc:
        with (
            tc.tile_pool(name="persist", bufs=1) as persist,
            tc.tile_pool(name="small", bufs=2) as small,
            tc.tile_pool(name="dram", bufs=1, space="DRAM") as dram,
        ):
            # ---- constants -------------------------------------------------
            ident = persist.tile([128, 128], BF16)
            masks.make_identity(nc, ident[:])

            # probe: ACT Copy/Identity bias exactness at C magnitude
            prb_in = small.tile([1, 4], F32)
            nc.vector.memset(prb_in[:], 0.37)
            prb_bias = small.tile([1, 1], F32)
            nc.vector.memset(prb_bias[:], C_MAGIC)
            prb_out = small.tile([1, 8], F32)
            nc.scalar.activation(prb_out[:, 0:4], prb_in[:], AF.Copy, bias=C_MAGIC)
            nc.scalar.activation(
                prb_out[:, 4:8], prb_in[:], AF.Identity, bias=prb_bias[:, 0:1]
            )
            nc.sync.dma_start(probe_d[:], prb_out[:])

            # bias broadcast: load into row 0 of bb, broadcast in place
            bb = persist.tile([128, do], F32)
            nc.sync.dma_start(bb[0:1, :], b_d[:])
            nc.gpsimd.partition_broadcast(bb[:], bb[0:1, :], channels=128)

            # ---- phase A pools + sharded mean(|W|) -------------------------
            qxT = persist.tile([128, kt, t], BF16)
            s_all = persist.tile([128, mt], F32)  # 127/amax per token
            c_all = persist.tile([128, mt], F32)  # scale_w*amax/127 per token
            scw = persist.tile([128, 1], F32)
            inv_w = persist.tile([128, 1], F32)
            with (
                tc.tile_pool(name="xphase", bufs=3) as xphase,
                tc.tile_pool(name="psumA", bufs=4, space="PSUM") as psumA,
            ):
                # mean(|W|) shard: full-row tiles (the f32 summation order is
                # deliberately row-sequential -- it lands scale_w within an
                # ulp of the jax reference's mean, avoiding ternary-weight
                # flips at round boundaries), abs-sum on DVE, then a 4-byte
                # AllReduce.  Own tile tag so the x pipeline starts at t=0.
                wsum_p = small.tile([128, sr // 128], F32)
                for i in range(sr // 128):
                    wti = xphase.tile([128, di], F32, tag="wmean", bufs=2, name="wti")
                    eng = nc.sync if i % 2 == 0 else nc.scalar
                    eng.dma_start(wti[:], ws_d[i * 128 : (i + 1) * 128, :])
                    nc.vector.tensor_reduce(
                        out=wsum_p[:, i : i + 1],
                        in_=wti[:],
                        axis=AX.X,
                        op=OP.add,
                        apply_absolute_value=True,
                    )
                wsum1 = small.tile([128, 1], F32)
                nc.vector.tensor_reduce(
                    out=wsum1[:], in_=wsum_p[:], axis=AX.X, op=OP.add
                )
                wsum_all = small.tile([128, 1], F32)
                nc.gpsimd.partition_all_reduce(
                    wsum_all[:], wsum1[:], channels=128,
                    reduce_op=bass_isa.ReduceOp.add,
                )
                cc_in = dram.tile([1, 1], F32)
                cc_out = dram.tile([1, 1], F32, addr_space="Shared")
                nc.sync.dma_start(cc_in[:], wsum_all[0:1, 0:1])
                nc.gpsimd.collective_compute(
                    "AllReduce",
                    OP.add,
                    replica_groups=[list(range(n_cores))],
                    ins=[cc_in[:]],
                    outs=[cc_out[:]],
                )
                tot = small.tile([1, 1], F32)
                nc.sync.dma_start(tot[:], cc_out[:])
                tot_b = small.tile([128, 1], F32)
                nc.gpsimd.partition_broadcast(tot_b[:], tot[:], channels=128)
                # scale_w = max(total / (D_IN*D_OUT), EPS); inv_w = 1/scale_w
                nc.vector.tensor_scalar(
                    scw[:], tot_b[:], 1.0 / (di * do), EPS, op0=OP.mult, op1=OP.max
                )
                nc.vector.reciprocal(inv_w[:], scw[:])

                # ---- phase A: activation quant + transpose ------------------
                for m in range(mt):
                    xtl = xphase.tile([128, di], F32, tag="x_in")
                    eng = nc.sync if m % 2 == 0 else nc.scalar
                    eng.dma_start(xtl[:], xs_d[m * 128 : (m + 1) * 128, :])
                    amax = xphase.tile([128, 1], F32, tag="amax")
                    nc.vector.tensor_reduce(
                        out=amax[:],
                        in_=xtl[:],
                        axis=AX.X,
                        op=OP.max,
                        apply_absolute_value=True,
                    )
                    amax_c = xphase.tile([128, 1], F32, tag="amax_c")
                    nc.vector.tensor_scalar(amax_c[:], amax[:], EPS, None, op0=OP.max)
                    rec = xphase.tile([128, 1], F32, tag="rec")
                    nc.vector.reciprocal(rec[:], amax_c[:])
                    nc.vector.tensor_scalar(
                        s_all[:, m : m + 1], rec[:], QMAX, None, op0=OP.mult
                    )
                    nc.vector.tensor_scalar(
                        c_all[:, m : m + 1],
                        amax_c[:],
                        scw[:, 0:1],
                        1.0 / QMAX,
                        op0=OP.mult,
                        op1=OP.mult,
                    )
                    # q_x = round(x*s): affine+round on ACT in place
                    # (probe-verified exact), then subtract C on DVE -> bf16
                    nc.scalar.activation(
                        xtl[:], xtl[:], AF.Copy,
                        bias=C_MAGIC, scale=s_all[:, m : m + 1],
                    )
                    qx = xphase.tile([128, di], BF16, tag="qx", bufs=2)
                    nc.vector.tensor_scalar(
                        qx[:], xtl[:], C_MAGIC, None, op0=OP.subtract
                    )
                    # transpose 128x128 blocks via PE into resident qxT
                    for j in range(kt):
                        pt = psumA.tile([128, 128], BF16, tag="pt")
                        last_tp = nc.tensor.transpose(
                            pt[:], qx[:, j * 128 : (j + 1) * 128], ident[:]
                        )
                        dst = qxT[:, j, m * 128 : (m + 1) * 128]
                        if j % 2 == 0:
                            nc.vector.tensor_copy(dst, pt[:])
                        else:
                            nc.scalar.copy(dst, pt[:])

            # ---- phase B: stream W, quantize, matmul, scale, store ---------
            # loop order n -> k -> m: each quantized W tile feeds all mt
            # token tiles back to back; W is read from HBM exactly once.
            with (
                tc.tile_pool(name="wpipe", bufs=6) as wpipe,
                tc.tile_pool(name="opipe", bufs=3) as opipe,
                tc.tile_pool(name="psumB", bufs=1, space="PSUM") as psumB,
            ):
                for n in range(nt):
                    # allocate high-m first: psum banks that overlap the
                    # (release-gated) phase-A transpose banks then belong to
                    # the LAST token tiles, whose own transposes finish last
                    # anyway -- early token tiles start unhindered.
                    pss = [None] * mt
                    for m in reversed(range(mt)):
                        pss[m] = psumB.tile([128, 512], F32, tag=f"mm{m}", name="ps")
                    for k in range(kt):
                        wtl = wpipe.tile([128, 512], F32, tag="w_in")
                        nc.gpsimd.dma_start(
                            wtl[:],
                            wt_d[k * 128 : (k + 1) * 128, n * 512 : (n + 1) * 512],
                        )
                        # u = W*inv_w on ACT (separate rounding step matches
                        # the reference's fl(W/scale)); then +C round and clip
                        # in the C-offset domain on DVE; sub C -> bf16
                        nc.scalar.activation(
                            wtl[:], wtl[:], AF.Copy, bias=0.0, scale=inv_w[:, 0:1]
                        )
                        nc.vector.tensor_scalar(
                            wtl[:], wtl[:], C_MAGIC, C_MAGIC + 1.0,
                            op0=OP.add, op1=OP.min,
                        )
                        qw = wpipe.tile([128, 512], BF16, tag="w_q")
                        nc.vector.tensor_scalar(
                            qw[:], wtl[:], C_MAGIC - 1.0, C_MAGIC,
                            op0=OP.max, op1=OP.subtract,
                        )
                        for m in range(mt):
                            mm = nc.tensor.matmul(
                                pss[m][:],
                                qxT[:, k, m * 128 : (m + 1) * 128],
                                qw[:],
                                start=(k == 0),
                                stop=(k == kt - 1),
                            )
                            if n == 0 and k == 0:
                                add_dep_helper(
                                    mm.ins, last_tp.ins, sync=False,
                                    reason="order all transposes before matmuls",
                                )
                    for m in range(mt):
                        ot = opipe.tile([128, 512], F32, tag="o_scaled")
                        nc.scalar.activation(
                            ot[:], pss[m][:], AF.Copy,
                            bias=0.0, scale=c_all[:, m : m + 1],
                        )
                        ot2 = opipe.tile([128, 512], F32, tag="o_final")
                        nc.gpsimd.tensor_tensor(
                            ot2[:], ot[:], bb[:, n * 512 : (n + 1) * 512], op=OP.add
                        )
                        nc.sync.dma_start(
                            out_d[m * 128 : (m + 1) * 128, n * 512 : (n + 1) * 512],
                            ot2[:],
                        )
    nc.compile()
    return nc


_PROGRAM = None


def _get_program():
    global _PROGRAM
    if _PROGRAM is None:
        _PROGRAM = build_bass()
    return _PROGRAM


def make_in_maps(x, W, b):
    """Shard full inputs into the 8 per-core input dicts."""
    x = np.ascontiguousarray(x, dtype=np.float32).reshape(T_FULL, D_IN)
    W = np.ascontiguousarray(W, dtype=np.float32)
    b = np.ascontiguousarray(b, dtype=np.float32).reshape(1, D_OUT)
    wt = np.ascontiguousarray(W.T)  # [in, out]
    in_maps = []
    for c in range(N_CORES):
        in_maps.append(
            {
                "xs": x[c * T : (c + 1) * T],
                "wt": wt,
                "wshard": np.ascontiguousarray(W[c * SR : (c + 1) * SR]),
                "bias": b,
            }
        )
    return in_maps


def kernel(x, W, b, trace=False, tmpdir=None):
    nc = _get_program()
    res = run_bass_kernel_spmd(
        nc,
        make_in_maps(x, W, b),
        core_ids=list(range(N_CORES)),
        trace=trace,
        tmpdir=tmpdir,
    )
    out = np.concatenate([res.results[c]["out"] for c in range(N_CORES)], axis=0)
    out = out.reshape(B, S, D_OUT)
    if trace:
        kernel.last_results = res
    return out


# revision 28
# speedup vs baseline: 1.0075x; 1.0075x over previous
"""BitLinear (BitNet b1.58-style) kernel for Trainium2, 8-core SPMD.

Reference computation (fp32):
    scale_w = max(mean(|W|), EPS)                       # scalar over all of W
    dq_w    = clip(round(W / scale_w), -1, 1) * scale_w # ternary weight
    amax_t  = max(max_j |x[t, j]|, EPS)                 # per token
    s_t     = 127 / amax_t
    dq_x    = round(x * s_t) / s_t                      # 8-bit absmax act quant
    out     = dq_x @ dq_w.T + b

Device strategy (data-parallel over tokens):
  * x is flattened to [8192, 4096] tokens and sharded 8 ways (1024/core).
  * W is transposed on the host once (layout choice) so each core can
    stream W.T = [in, out] tiles with K on partitions, and replicated.
  * The |W| mean reduction is sharded 8 ways (each core reduces 512 rows
    of W) and combined with a 4-byte AllReduce.
  * The matmul runs in bf16 on the PE array: q_x in [-127,127] and
    q_w in {-1,0,1} are bf16-exact, and fp32 PSUM accumulation of 4096
    products bounded by 127 is exact (< 2^24). The scales are folded
    into a per-token multiplier applied on PSUM eviction:
        out[t, :] = (q_x @ q_w.T)[t, :] * (scale_w * amax_t / 127) + b
  * Rounding uses the fp32 magic-constant trick (v + 1.5*2^23) - 1.5*2^23,
    round-to-nearest-even, matching jnp.round.  For W the multiply by
    1/scale_w and the +C rounding are SEPARATE instructions (ACT then
    DVE) so the double rounding matches the reference's fl(W/scale)
    then round() exactly; a fused FMA would single-round and flip a few
    ternary weights at the 0.5/1.5 boundaries.
"""

import numpy as np

from concourse import bacc, bass_isa, masks, mybir, tile
from concourse.tile import add_dep_helper
from concourse.bass_utils import run_bass_kernel_spmd

F32 = mybir.dt.float32
BF16 = mybir.dt.bfloat16
AX = mybir.AxisListType
OP = mybir.AluOpType
AF = mybir.ActivationFunctionType

EPS = 1e-6
QMAX = 127.0
C_MAGIC = 1.5 * 2.0**23  # fp32 RNE rounding constant

N_CORES = 8
B, S, D_IN, D_OUT = 4, 2048, 4096, 4096
T_FULL = B * S  # 8192 tokens
T = T_FULL // N_CORES  # 1024 tokens per core
SR = D_OUT // N_CORES  # 512 W rows per core for the |W| mean


def build_bass(t=T, di=D_IN, do=D_OUT, sr=SR, n_cores=N_CORES):
    """Emit the per-core SPMD program. All cores run the same code on
    their own shard; the only cross-core op is a 4-byte AllReduce."""
    assert t % 128 == 0 and di % 512 == 0 and do % 512 == 0 and sr % 128 == 0
    mt = t // 128  # token tiles
    kt = di // 128  # contraction tiles
    nt = do // 512  # output-column blocks
    assert mt <= 8, "one PSUM bank per token tile"

    nc = bacc.Bacc(None)
    xs_d = nc.dram_tensor("xs", [t, di], F32, kind="ExternalInput")
    wt_d = nc.dram_tensor("wt", [di, do], F32, kind="ExternalInput")
    ws_d = nc.dram_tensor("wshard", [sr, di], F32, kind="ExternalInput")
    b_d = nc.dram_tensor("bias", [1, do], F32, kind="ExternalInput")
    out_d = nc.dram_tensor("out", [t, do], F32, kind="ExternalOutput")
    probe_d = nc.dram_tensor("probe", [1, 8], F32, kind="ExternalOutput")

    with tile.TileContext(nc) as tc:
        with (
            tc.tile_pool(name="persist", bufs=1) as persist,
            tc.tile_pool(name="small", bufs=2) as small,
            tc.tile_pool(name="dram", bufs=1, space="DRAM") as dram,
        ):
            # ---- constants -------------------------------------------------
            ident = persist.tile([128, 128], BF16)
            masks.make_identity(nc, ident[:])

            # probe: ACT Copy/Identity bias exactness at C magnitude
            prb_in = small.tile([1, 4], F32)
            nc.vector.memset(prb_in[:], 0.37)
            prb_bias = small.tile([1, 1], F32)
            nc.vector.memset(prb_bias[:], C_MAGIC)
            prb_out = small.tile([1, 8], F32)
            nc.scalar.activation(prb_out[:, 0:4], prb_in[:], AF.Copy, bias=C_MAGIC)
            nc.scalar.activation(
                prb_out[:, 4:8], prb_in[:], AF.Identity, bias=prb_bias[:, 0:1]
            )
            nc.sync.dma_start(probe_d[:], prb_out[:])

            # bias broadcast: load into row 0 of bb, broadcast in place
            bb = persist.tile([128, do], F32)
            nc.sync.dma_start(bb[0:1, :], b_d[:])
            nc.gpsimd.partition_broadcast(bb[:], bb[0:1, :], channels=128)

            # ---- phase A pools + sharded mean(|W|) -------------------------
            qxT = persist.tile([128, kt, t], BF16)
            s_all = persist.tile([128, mt], F32)  # 127/amax per token
            c_all = persist.tile([128, mt], F32)  # scale_w*amax/127 per token
            scw = persist.tile([128, 1], F32)
            inv_w = persist.tile([128, 1], F32)
            with (
                tc.tile_pool(name="xphase", bufs=3) as xphase,
                tc.tile_pool(name="psumA", bufs=4, space="PSUM") as psumA,
            ):
                # mean(|W|) shard: full-row tiles (the f32 summation order is
                # deliberately row-sequential -- it lands scale_w within an
                # ulp of the jax reference's mean, avoiding ternary-weight
                # flips at round boundaries), abs-sum on DVE, then a 4-byte
                # AllReduce.  Own tile tag so the x pipeline starts at t=0.
                wsum_p = small.tile([128, sr // 128], F32)
                for i in range(sr // 128):
                    wti = xphase.tile([128, di], F32, tag="wmean", bufs=2, name="wti")
                    eng = nc.sync if i % 2 == 0 else nc.scalar
                    eng.dma_start(wti[:], ws_d[i * 128 : (i + 1) * 128, :])
                    nc.vector.tensor_reduce(
                        out=wsum_p[:, i : i + 1],
                        in_=wti[:],
                        axis=AX.X,
                        op=OP.add,
                        apply_absolute_value=True,
                    )
                wsum1 = small.tile([128, 1], F32)
                nc.vector.tensor_reduce(
                    out=wsum1[:], in_=wsum_p[:], axis=AX.X, op=OP.add
                )
                wsum_all = small.tile([128, 1], F32)
                nc.gpsimd.partition_all_reduce(
                    wsum_all[:], wsum1[:], channels=128,
                    reduce_op=bass_isa.ReduceOp.add,
                )
                cc_in = dram.tile([1, 1], F32)
                cc_out = dram.tile([1, 1], F32, addr_space="Shared")
                nc.sync.dma_start(cc_in[:], wsum_all[0:1, 0:1])
                nc.gpsimd.collective_compute(
                    "AllReduce",
                    OP.add,
                    replica_groups=[list(range(n_cores))],
                    ins=[cc_in[:]],
                    outs=[cc_out[:]],
                )
                tot = small.tile([1, 1], F32)
                nc.sync.dma_start(tot[:], cc_out[:])
                tot_b = small.tile([128, 1], F32)
                nc.gpsimd.partition_broadcast(tot_b[:], tot[:], channels=128)
                # scale_w = max(total / (D_IN*D_OUT), EPS); inv_w = 1/scale_w
                nc.vector.tensor_scalar(
                    scw[:], tot_b[:], 1.0 / (di * do), EPS, op0=OP.mult, op1=OP.max
                )
                nc.vector.reciprocal(inv_w[:], scw[:])

                # ---- phase A: activation quant + transpose ------------------
                for m in range(mt):
                    xtl = xphase.tile([128, di], F32, tag="x_in")
                    eng = nc.sync if m % 2 == 0 else nc.scalar
                    eng.dma_start(xtl[:], xs_d[m * 128 : (m + 1) * 128, :])
                    amax = xphase.tile([128, 1], F32, tag="amax")
                    nc.vector.tensor_reduce(
                        out=amax[:],
                        in_=xtl[:],
                        axis=AX.X,
                        op=OP.max,
                        apply_absolute_value=True,
                    )
                    amax_c = xphase.tile([128, 1], F32, tag="amax_c")
                    nc.vector.tensor_scalar(amax_c[:], amax[:], EPS, None, op0=OP.max)
                    rec = xphase.tile([128, 1], F32, tag="rec")
                    nc.vector.reciprocal(rec[:], amax_c[:])
                    nc.vector.tensor_scalar(
                        s_all[:, m : m + 1], rec[:], QMAX, None, op0=OP.mult
                    )
                    nc.vector.tensor_scalar(
                        c_all[:, m : m + 1],
                        amax_c[:],
                        scw[:, 0:1],
                        1.0 / QMAX,
                        op0=OP.mult,
                        op1=OP.mult,
                    )
                    # q_x = round(x*s): affine+round on ACT in place
                    # (probe-verified exact), then subtract C on DVE -> bf16
                    last_round = nc.scalar.activation(
                        xtl[:], xtl[:], AF.Copy,
                        bias=C_MAGIC, scale=s_all[:, m : m + 1],
                    )
                    qx = xphase.tile([128, di], BF16, tag="qx", bufs=2)
                    last_qxsub = nc.vector.tensor_scalar(
                        qx[:], xtl[:], C_MAGIC, None, op0=OP.subtract
                    )
                    # transpose 128x128 blocks via PE into resident qxT
                    for j in range(kt):
                        pt = psumA.tile([128, 128], BF16, tag="pt")
                        last_tp = nc.tensor.transpose(
                            pt[:], qx[:, j * 128 : (j + 1) * 128], ident[:]
                        )
                        dst = qxT[:, j, m * 128 : (m + 1) * 128]
                        if j % 2 == 0:
                            nc.vector.tensor_copy(dst, pt[:])
                        else:
                            nc.scalar.copy(dst, pt[:])

            # ---- phase B: stream W, quantize, matmul, scale, store ---------
            # loop order n -> k -> m: each quantized W tile feeds all mt
            # token tiles back to back; W is read from HBM exactly once.
            with (
                tc.tile_pool(name="wpipe", bufs=6) as wpipe,
                tc.tile_pool(name="opipe", bufs=3) as opipe,
                tc.tile_pool(name="psumB", bufs=1, space="PSUM") as psumB,
            ):
                for n in range(nt):
                    # allocate high-m first: psum banks that overlap the
                    # (release-gated) phase-A transpose banks then belong to
                    # the LAST token tiles, whose own transposes finish last
                    # anyway -- early token tiles start unhindered.
                    pss = [None] * mt
                    for m in reversed(range(mt)):
                        pss[m] = psumB.tile([128, 512], F32, tag=f"mm{m}", name="ps")
                    for k in range(kt):
                        wtl = wpipe.tile([128, 512], F32, tag="w_in")
                        nc.gpsimd.dma_start(
                            wtl[:],
                            wt_d[k * 128 : (k + 1) * 128, n * 512 : (n + 1) * 512],
                        )
                        # u = W*inv_w on ACT (separate rounding step matches
                        # the reference's fl(W/scale)); then +C round and clip
                        # in the C-offset domain on DVE; sub C -> bf16
                        wa = nc.scalar.activation(
                            wtl[:], wtl[:], AF.Copy, bias=0.0, scale=inv_w[:, 0:1]
                        )
                        wc = nc.vector.tensor_scalar(
                            wtl[:], wtl[:], C_MAGIC, C_MAGIC + 1.0,
                            op0=OP.add, op1=OP.min,
                        )
                        if n == 0 and k == 0:
                            add_dep_helper(
                                wa.ins, last_round.ins, sync=False,
                                reason="phase-A ACT work before W affine",
                            )
                            add_dep_helper(
                                wc.ins, last_qxsub.ins, sync=False,
                                reason="phase-A DVE work before W clip",
                            )
                        qw = wpipe.tile([128, 512], BF16, tag="w_q")
                        nc.vector.tensor_scalar(
                            qw[:], wtl[:], C_MAGIC - 1.0, C_MAGIC,
                            op0=OP.max, op1=OP.subtract,
                        )
                        for m in range(mt):
                            mm = nc.tensor.matmul(
                                pss[m][:],
                                qxT[:, k, m * 128 : (m + 1) * 128],
                                qw[:],
                                start=(k == 0),
                                stop=(k == kt - 1),
                            )
                            if n == 0 and k == 0:
                                add_dep_helper(
                                    mm.ins, last_tp.ins, sync=False,
                                    reason="order all transposes before matmuls",
                                )
                    for m in range(mt):
                        ot = opipe.tile([128, 512], F32, tag="o_scaled")
                        nc.scalar.activation(
                            ot[:], pss[m][:], AF.Copy,
                            bias=0.0, scale=c_all[:, m : m + 1],
                        )
                        ot2 = opipe.tile([128, 512], F32, tag="o_final")
                        nc.gpsimd.tensor_tensor(
                            ot2[:], ot[:], bb[:, n * 512 : (n + 1) * 512], op=OP.add
                        )
                        nc.sync.dma_start(
                            out_d[m * 128 : (m + 1) * 128, n * 512 : (n + 1) * 512],
                            ot2[:],
                        )
    nc.compile()
    return nc


_PROGRAM = None


def _get_program():
    global _PROGRAM
    if _PROGRAM is None:
        _PROGRAM = build_bass()
    return _PROGRAM


def make_in_maps(x, W, b):
    """Shard full inputs into the 8 per-core input dicts."""
    x = np.ascontiguousarray(x, dtype=np.float32).reshape(T_FULL, D_IN)
    W = np.ascontiguousarray(W, dtype=np.float32)
    b = np.ascontiguousarray(b, dtype=np.float32).reshape(1, D_OUT)
    wt = np.ascontiguousarray(W.T)  # [in, out]
    in_maps = []
    for c in range(N_CORES):
        in_maps.append(
            {
                "xs": x[c * T : (c + 1) * T],
                "wt": wt,
                "wshard": np.ascontiguousarray(W[c * SR : (c + 1) * SR]),
                "bias": b,
            }
        )
    return in_maps


def kernel(x, W, b, trace=False, tmpdir=None):
    nc = _get_program()
    res = run_bass_kernel_spmd(
        nc,
        make_in_maps(x, W, b),
        core_ids=list(range(N_CORES)),
        trace=trace,
        tmpdir=tmpdir,
    )
    out = np.concatenate([res.results[c]["out"] for c in range(N_CORES)], axis=0)
    out = out.reshape(B, S, D_OUT)
    if trace:
        kernel.last_results = res
    return out
